# revision 1
# baseline (speedup 1.0000x reference)
"""Multi-head attention (B=2, S=4096, D=768, H=12) on 8 trn2 NeuronCores.

Sharding: data-parallel over batch (2) x tensor-parallel over head groups (4):
core c -> batch c//4, heads [3*(c%4), 3*(c%4)+3). Each core projects Q/K/V for
its 3 heads (column-sliced W_q/W_k/W_v), runs flash-style attention in the
transposed (scores^T) domain, applies its row slice of W_o, and a 4-way
ReduceScatter sums the partial outputs, leaving each core with its sequence
quarter of the final output.

All matmuls run as float32r (full-rate fp32 PE mode, ~1.4e-4 rel rounding).
Softmax skips max-subtraction (scores are provably small: |s|<~2.5) and the
denominator is produced by an extra ones-column in the attn@V stationary.
"""
import contextlib
import ctypes
import sys
import types

import numpy as np

# ---------------------------------------------------------------------------
# NTFF profile hook (image's antenv lacks axon_hooks; install shim so
# run_bass_kernel_spmd(trace=True) can capture exec_time_ns).
# ---------------------------------------------------------------------------
def _install_ntff_hook():
    try:
        from antenv.axon_hooks import get_axon_ntff_profile_hook  # noqa: F401
        return
    except ImportError:
        pass
    import antenv

    mod = types.ModuleType("antenv.axon_hooks")
    _state = {"hook": None}
    mod.set_axon_ntff_profile_hook = lambda h: _state.__setitem__("hook", h)
    mod.get_axon_ntff_profile_hook = lambda: _state["hook"]
    sys.modules["antenv.axon_hooks"] = mod
    antenv.axon_hooks = mod

    try:
        lib = ctypes.CDLL("/opt/axon/libaxon_pjrt.so")
    except OSError:
        return
    if not hasattr(lib, "axon_start_nrt_profile"):
        return
    lib.axon_start_nrt_profile.argtypes = [ctypes.POINTER(ctypes.c_int64), ctypes.c_size_t]
    lib.axon_start_nrt_profile.restype = ctypes.c_int64
    lib.axon_stop_nrt_profile.argtypes = [ctypes.c_char_p]
    lib.axon_stop_nrt_profile.restype = ctypes.c_int64

    @contextlib.contextmanager
    def _hook(output_dir, device_ids):
        import jax

        jax.devices()
        if device_ids:
            ids = (ctypes.c_int64 * len(device_ids))(*device_ids)
            rc = lib.axon_start_nrt_profile(ids, len(device_ids))
        else:
            rc = lib.axon_start_nrt_profile(None, 0)
        if rc != 0:
            raise RuntimeError(f"axon_start_nrt_profile rc={rc}")
        try:
            yield
        finally:
            n = lib.axon_stop_nrt_profile(str(output_dir).encode())
            print(f"ntff profile: {n} file(s) -> {output_dir}", file=sys.stderr)

    mod.set_axon_ntff_profile_hook(_hook)


_install_ntff_hook()

import concourse.bass as bass  # noqa: E402
import concourse.tile as tile  # noqa: E402
from concourse import bacc, bass_utils, mybir  # noqa: E402
from concourse.masks import make_identity  # noqa: E402

f32 = mybir.dt.float32
f32r = mybir.dt.float32r
AF = mybir.ActivationFunctionType

B, S, D = 2, 4096, 768
H, DH = 12, 64
NCORES = 8
HPC = 3               # heads per core
E = HPC * DH          # 192: per-core projection width
EP = 256              # padded V projection width (N>=256 keeps fp32r at full rate)
NQC = 4               # q chunks of 1024
QC = S // NQC         # 1024
NST = S // 128        # 32 s-tiles
NCH = S // 512        # 8 projection chunks


def _build_nc():
    nc = bacc.Bacc("TRN2", target_bir_lowering=False, debug=False, num_devices=NCORES)
    xq = nc.dram_tensor("xq", [S, D], f32, kind="ExternalInput").ap()
    xk = nc.dram_tensor("xk", [S, D], f32, kind="ExternalInput").ap()
    xv = nc.dram_tensor("xv", [S, D], f32, kind="ExternalInput").ap()
    wqT = nc.dram_tensor("wqT", [D, E], f32, kind="ExternalInput").ap()
    wkT = nc.dram_tensor("wkT", [D, E], f32, kind="ExternalInput").ap()
    wvT = nc.dram_tensor("wvT", [D, EP], f32, kind="ExternalInput").ap()
    woT = nc.dram_tensor("woT", [E, D], f32, kind="ExternalInput").ap()
    y = nc.dram_tensor("y", [S // 4, D], f32, kind="ExternalOutput").ap()

    with tile.TileContext(nc) as tc:
        _body(tc, xq, xk, xv, wqT, wkT, wvT, woT, y)
    nc.compile()
    return nc


def _body(tc, xq, xk, xv, wqT, wkT, wvT, woT, y):
    nc = tc.nc
    with contextlib.ExitStack() as ctx:
        const = ctx.enter_context(tc.tile_pool(name="const", bufs=1))
        big = ctx.enter_context(tc.tile_pool(name="big", bufs=1))
        wstage_p = ctx.enter_context(tc.tile_pool(name="wstage", bufs=1))
        xload_p = ctx.enter_context(tc.tile_pool(name="xload", bufs=4))
        strip_p = ctx.enter_context(tc.tile_pool(name="strip", bufs=7))
        expt_p = ctx.enter_context(tc.tile_pool(name="expt", bufs=2))
        small_p = ctx.enter_context(tc.tile_pool(name="small", bufs=2))
        ysb_p = ctx.enter_context(tc.tile_pool(name="ysb", bufs=2))
        ps_s = ctx.enter_context(tc.tile_pool(name="ps_s", bufs=2, space="PSUM"))
        ps_o = ctx.enter_context(tc.tile_pool(name="ps_o", bufs=2, space="PSUM"))
        dram = ctx.enter_context(tc.tile_pool(name="dram", bufs=1, space="DRAM"))

        # ---- constants ----
        ident = const.tile([128, 128], f32)
        make_identity(nc, ident[:])
        ones_f = const.tile([128, 1], f32)
        nc.any.memset(ones_f[:], 1.0)
        ones_r = const.tile([1, 64], f32r)   # bcast matmul stationary
        onesf_row = const.tile([1, 64], f32)
        nc.any.memset(onesf_row[:], 1.0)
        nc.vector.tensor_copy(ones_r[:], onesf_row[:])
        ones3 = const.tile([128, HPC], f32)  # Vones ones-columns source
        nc.any.memset(ones3[:], 1.0)

        # ---- persistent activations (f32r) ----
        QT0 = big.tile([128, S], f32r)   # heads 0,1 of this core's group: Q^T
        QT1 = big.tile([64, S], f32r)    # head 2: Q^T
        KT0 = big.tile([128, S], f32r)   # heads 0,1: K^T
        KT1p = big.tile([128, S], f32r)  # rows 0-63: head2 K^T ; rows 64-127: head2 outT (normalized)
        OT0 = big.tile([128, S], f32r)   # normalized outT heads 0,1
        VON = big.tile([128, NST * HPC * 65], f32r)  # per s-tile: [V_h|1] x3 interleaved

        # ---- weights -> SBUF f32r ----
        wq_r = big.tile([128, 6 * E], f32r)
        wk_r = big.tile([128, 6 * E], f32r)
        wv_r = big.tile([128, 6 * EP], f32r)
        wo_r0 = big.tile([128, D], f32r)           # woT rows 0-127
        wo_r1 = big.tile([128, D], f32r)           # rows 64-127 hold woT rows 128-191

        for w_dram, w_sb, width in ((wqT, wq_r, E), (wkT, wk_r, E), (wvT, wv_r, EP)):
            stage = wstage_p.tile([128, 6 * width], f32, tag="wstage")
            for j in range(6):
                nc.sync.dma_start(stage[:, j * width:(j + 1) * width],
                                  w_dram[j * 128:(j + 1) * 128, :])
            nc.vector.tensor_copy(w_sb[:], stage[:])
        stage = wstage_p.tile([128, D], f32, tag="wstage")
        nc.sync.dma_start(stage[:], woT[0:128, :])
        nc.vector.tensor_copy(wo_r0[:], stage[:])
        stage = wstage_p.tile([128, D], f32, tag="wstage")
        nc.sync.dma_start(stage[64:128, :], woT[128:192, :])
        nc.vector.tensor_copy(wo_r1[64:128, :], stage[64:128, :])

        # ---- phase 1: transpose inputs + projections ----
        def proj_qk(x_dram, w_sb, dst0, dst1):
            # dst0 [128, S] (e rows 0-127), dst1 [64, S] (e rows 128-191)
            for c in range(NCH):
                strips = []
                xt = []
                for st in range(4):
                    t = xload_p.tile([128, D], f32, tag="xload")
                    nc.sync.dma_start(t[:], x_dram[c * 512 + st * 128:c * 512 + (st + 1) * 128, :])
                    xt.append(t)
                for dt in range(6):
                    tp = ps_s.tile([128, 512], f32, tag="s")
                    for st in range(4):
                        nc.tensor.transpose(tp[:, st * 128:(st + 1) * 128],
                                            xt[st][:, dt * 128:(dt + 1) * 128], ident[:])
                    sb = strip_p.tile([128, 512], f32r, tag="strip")
                    nc.vector.tensor_copy(sb[:], tp[:])
                    strips.append(sb)
                for ep, (lo, sz) in enumerate(((0, 128), (128, 64))):
                    pp = ps_o.tile([128, 512], f32, tag="o")
                    for dt in range(6):
                        nc.tensor.matmul(pp[0:sz, :],
                                         wq_like_slice(w_sb, dt, lo, sz),
                                         strips[dt][:],
                                         start=(dt == 0), stop=(dt == 5))
                    dst = dst0 if ep == 0 else dst1
                    nc.vector.tensor_copy(dst[0:sz, c * 512:(c + 1) * 512], pp[0:sz, :])

        def wq_like_slice(w_sb, dt, lo, sz):
            return w_sb[:, dt * E + lo:dt * E + lo + sz]

        proj_qk(xq, wq_r, QT0, QT1)
        proj_qk(xk, wk_r, KT0, KT1p)

        # V: natural layout with interleaved ones columns
        for c in range(NCH):
            strips = []
            xt = []
            for st in range(4):
                t = xload_p.tile([128, D], f32, tag="xload")
                nc.sync.dma_start(t[:], xv[c * 512 + st * 128:c * 512 + (st + 1) * 128, :])
                xt.append(t)
            for dt in range(6):
                tp = ps_s.tile([128, 512], f32, tag="s")
                for st in range(4):
                    nc.tensor.transpose(tp[:, st * 128:(st + 1) * 128],
                                        xt[st][:, dt * 128:(dt + 1) * 128], ident[:])
                sb = strip_p.tile([128, 512], f32r, tag="strip")
                nc.vector.tensor_copy(sb[:], tp[:])
                strips.append(sb)
            for st in range(4):
                pp = ps_o.tile([128, EP], f32, tag="o")
                for dt in range(6):
                    nc.tensor.matmul(pp[:], strips[dt][:, st * 128:(st + 1) * 128],
                                     wv_r[:, dt * EP:(dt + 1) * EP],
                                     start=(dt == 0), stop=(dt == 5))
                gst = c * 4 + st
                von = VON[:, gst * HPC * 65:(gst + 1) * HPC * 65]
                v3 = von.rearrange("p (h c) -> p h c", c=65)
                nc.vector.tensor_copy(v3[:, :, 0:64],
                                      pp[:, 0:E].rearrange("p (h c) -> p h c", c=64))
                nc.vector.tensor_copy(v3[:, :, 64:65],
                                      ones3[:].rearrange("p (h c) -> p h c", c=1))

        # ---- phase 2: attention (transposed domain) ----
        def head_aps(h):
            if h == 0:
                return KT0[0:64, :], QT0[0:64, :]
            if h == 1:
                return KT0[64:128, :], QT0[64:128, :]
            return KT1p[0:64, :], QT1[0:64, :]

        for qc in range(NQC):
            q0, q1 = qc * QC, (qc + 1) * QC
            for h in range(HPC):
                KT_h, QT_h = head_aps(h)
                po = ps_o.tile([65, QC], f32, tag="o")
                for t in range(NST):
                    pscore = ps_s.tile([128, QC], f32, tag="s")
                    for half in range(2):
                        nc.tensor.matmul(
                            pscore[:, half * 512:(half + 1) * 512],
                            KT_h[:, t * 128:(t + 1) * 128],
                            QT_h[:, q0 + half * 512:q0 + (half + 1) * 512],
                            start=True, stop=True)
                    et = expt_p.tile([128, QC], f32r, tag="expt")
                    nc.scalar.activation(et[:], pscore[:], AF.Exp, scale=0.125)
                    von = VON[:, (t * HPC + h) * 65:(t * HPC + h + 1) * 65]
                    for half in range(2):
                        nc.tensor.matmul(
                            po[:, half * 512:(half + 1) * 512],
                            von, et[:, half * 512:(half + 1) * 512],
                            start=(t == 0), stop=(t == NST - 1))
                # normalize: outT[d, q] / denom[q]
                rc = small_p.tile([1, QC], f32r, tag="recip")
                with nc.allow_low_precision(reason="softmax denominator"):
                    nc.vector.reciprocal(rc[:], po[64:65, :])
                pb = ps_s.tile([128, QC], f32, tag="s")
                for half in range(2):
                    nc.tensor.matmul(pb[0:64, half * 512:(half + 1) * 512],
                                     ones_r[:], rc[:, half * 512:(half + 1) * 512],
                                     start=True, stop=True)
                bs = small_p.tile([64, QC], f32, tag="bcast")
                nc.vector.tensor_copy(bs[:], pb[0:64, :])
                nrm = small_p.tile([64, QC], f32r, tag="nrm")
                nc.vector.tensor_mul(nrm[:], po[0:64, :], bs[:])
                # move into packed destination (partition shift needs DMA)
                if h == 0:
                    nc.vector.tensor_copy(OT0[0:64, q0:q1], nrm[:])
                elif h == 1:
                    nc.sync.dma_start(OT0[64:128, q0:q1], nrm[:])
                else:
                    nc.sync.dma_start(KT1p[64:128, q0:q1], nrm[:])

        # ---- phase 3: W_o + ReduceScatter ----
        rs_in = dram.tile([S, D], f32)
        rs_out = dram.tile([S // 4, D], f32)
        for st in range(NST):
            py = ps_o.tile([128, D], f32, tag="o")
            for e0, esz in ((0, 512), (512, 256)):
                nc.tensor.matmul(py[:, e0:e0 + esz],
                                 OT0[:, st * 128:(st + 1) * 128],
                                 wo_r0[:, e0:e0 + esz], start=True, stop=False)
                nc.tensor.matmul(py[:, e0:e0 + esz],
                                 KT1p[64:128, st * 128:(st + 1) * 128],
                                 wo_r1[64:128, e0:e0 + esz], start=False, stop=True)
            ys = ysb_p.tile([128, D], f32, tag="ysb")
            nc.vector.tensor_copy(ys[:], py[:])
            nc.sync.dma_start(rs_in[st * 128:(st + 1) * 128, :], ys[:])

        nc.gpsimd.collective_compute(
            "ReduceScatter",
            mybir.AluOpType.add,
            replica_groups=[[0, 1, 2, 3], [4, 5, 6, 7]],
            ins=[rs_in.opt()],
            outs=[rs_out.opt()],
        )
        nc.sync.dma_start(y[:], rs_out[:])


_NC_CACHE = None


def _get_nc():
    global _NC_CACHE
    if _NC_CACHE is None:
        _NC_CACHE = _build_nc()
    return _NC_CACHE


def _make_in_maps(query, key, value, W_q, W_k, W_v, W_o):
    query = np.asarray(query, dtype=np.float32)
    key = np.asarray(key, dtype=np.float32)
    value = np.asarray(value, dtype=np.float32)
    wq_t = np.ascontiguousarray(np.asarray(W_q, np.float32).T)  # [d_in, e_out]
    wk_t = np.ascontiguousarray(np.asarray(W_k, np.float32).T)
    wv_t = np.ascontiguousarray(np.asarray(W_v, np.float32).T)
    wo_t = np.ascontiguousarray(np.asarray(W_o, np.float32).T)  # [d_in(heads), e_out]
    in_maps = []
    for c in range(NCORES):
        b, g = c // 4, c % 4
        sl = slice(g * E, (g + 1) * E)
        wv_pad = np.zeros((D, EP), np.float32)
        wv_pad[:, 0:E] = wv_t[:, sl]
        in_maps.append({
            "xq": np.ascontiguousarray(query[b]),
            "xk": np.ascontiguousarray(key[b]),
            "xv": np.ascontiguousarray(value[b]),
            "wqT": np.ascontiguousarray(wq_t[:, sl]),
            "wkT": np.ascontiguousarray(wk_t[:, sl]),
            "wvT": wv_pad,
            "woT": np.ascontiguousarray(wo_t[sl, :]),
        })
    return in_maps


def run(in_maps, trace=False):
    nc = _get_nc()
    return bass_utils.run_bass_kernel_spmd(
        nc, in_maps, core_ids=list(range(NCORES)), trace=trace)


def kernel(**inputs):
    in_maps = _make_in_maps(**inputs)
    res = run(in_maps)
    out = np.empty((B, S, D), np.float32)
    for c in range(NCORES):
        b, g = c // 4, c % 4
        out[b, g * (S // 4):(g + 1) * (S // 4)] = res.results[c]["y"]
    return out


# revision 2
# speedup vs baseline: 1.0840x; 1.0840x over previous
"""Multi-head attention (B=2, S=4096, D=768, H=12) on 8 trn2 NeuronCores.

Sharding: data-parallel over batch (2) x tensor-parallel over head groups (4):
core c -> batch c//4, heads [3*(c%4), 3*(c%4)+3). Each core projects Q/K/V for
its 3 heads (column-sliced W_q/W_k/W_v), runs flash-style attention in the
transposed (scores^T) domain, applies its row slice of W_o, and a 4-way
ReduceScatter sums the partial outputs, leaving each core with its sequence
quarter of the final output.

All matmul operands are fp16 (1 cyc/row on the PE with fast weight loads;
~2.4e-4 rounding) with fp32 PSUM accumulation. Softmax skips max-subtraction
(scores are provably small: |s|<~2.5) and the denominator is produced by an
extra ones-column in the attn@V stationary.
"""
import contextlib
import ctypes
import sys
import types

import numpy as np

# ---------------------------------------------------------------------------
# NTFF profile hook (image's antenv lacks axon_hooks; install shim so
# run_bass_kernel_spmd(trace=True) can capture exec_time_ns).
# ---------------------------------------------------------------------------
def _install_ntff_hook():
    try:
        from antenv.axon_hooks import get_axon_ntff_profile_hook  # noqa: F401
        return
    except ImportError:
        pass
    import antenv

    mod = types.ModuleType("antenv.axon_hooks")
    _state = {"hook": None}
    mod.set_axon_ntff_profile_hook = lambda h: _state.__setitem__("hook", h)
    mod.get_axon_ntff_profile_hook = lambda: _state["hook"]
    sys.modules["antenv.axon_hooks"] = mod
    antenv.axon_hooks = mod

    try:
        lib = ctypes.CDLL("/opt/axon/libaxon_pjrt.so")
    except OSError:
        return
    if not hasattr(lib, "axon_start_nrt_profile"):
        return
    lib.axon_start_nrt_profile.argtypes = [ctypes.POINTER(ctypes.c_int64), ctypes.c_size_t]
    lib.axon_start_nrt_profile.restype = ctypes.c_int64
    lib.axon_stop_nrt_profile.argtypes = [ctypes.c_char_p]
    lib.axon_stop_nrt_profile.restype = ctypes.c_int64

    @contextlib.contextmanager
    def _hook(output_dir, device_ids):
        import jax

        jax.devices()
        if device_ids:
            ids = (ctypes.c_int64 * len(device_ids))(*device_ids)
            rc = lib.axon_start_nrt_profile(ids, len(device_ids))
        else:
            rc = lib.axon_start_nrt_profile(None, 0)
        if rc != 0:
            raise RuntimeError(f"axon_start_nrt_profile rc={rc}")
        try:
            yield
        finally:
            n = lib.axon_stop_nrt_profile(str(output_dir).encode())
            print(f"ntff profile: {n} file(s) -> {output_dir}", file=sys.stderr)

    mod.set_axon_ntff_profile_hook(_hook)


_install_ntff_hook()

import concourse.bass as bass  # noqa: E402
import concourse.tile as tile  # noqa: E402
from concourse import bacc, bass_utils, mybir  # noqa: E402
from concourse.masks import make_identity  # noqa: E402

f32 = mybir.dt.float32
f16 = mybir.dt.float16
AF = mybir.ActivationFunctionType

B, S, D = 2, 4096, 768
H, DH = 12, 64
NCORES = 8
HPC = 3               # heads per core
E = HPC * DH          # 192: per-core projection width
EP = 256              # padded V projection width (N>=256 keeps fp32r at full rate)
NQC = 4               # q chunks of 1024
QC = S // NQC         # 1024
NST = S // 128        # 32 s-tiles
NCH = S // 512        # 8 projection chunks


def _build_nc():
    nc = bacc.Bacc("TRN2", target_bir_lowering=False, debug=False, num_devices=NCORES)
    xq = nc.dram_tensor("xq", [S, D], f32, kind="ExternalInput").ap()
    xk = nc.dram_tensor("xk", [S, D], f32, kind="ExternalInput").ap()
    xv = nc.dram_tensor("xv", [S, D], f32, kind="ExternalInput").ap()
    wqT = nc.dram_tensor("wqT", [D, E], f32, kind="ExternalInput").ap()
    wkT = nc.dram_tensor("wkT", [D, E], f32, kind="ExternalInput").ap()
    wvT = nc.dram_tensor("wvT", [D, EP], f32, kind="ExternalInput").ap()
    woT = nc.dram_tensor("woT", [E, D], f32, kind="ExternalInput").ap()
    y = nc.dram_tensor("y", [S // 4, D], f32, kind="ExternalOutput").ap()

    with tile.TileContext(nc) as tc:
        _body(tc, xq, xk, xv, wqT, wkT, wvT, woT, y)
    nc.compile()
    return nc


def _body(tc, xq, xk, xv, wqT, wkT, wvT, woT, y):
    nc = tc.nc
    with contextlib.ExitStack() as ctx:
        const = ctx.enter_context(tc.tile_pool(name="const", bufs=1))
        big = ctx.enter_context(tc.tile_pool(name="big", bufs=1))
        wstage_p = ctx.enter_context(tc.tile_pool(name="wstage", bufs=1))
        xload_p = ctx.enter_context(tc.tile_pool(name="xload", bufs=4))
        strip_p = ctx.enter_context(tc.tile_pool(name="strip", bufs=7))
        expt_p = ctx.enter_context(tc.tile_pool(name="expt", bufs=2))
        small_p = ctx.enter_context(tc.tile_pool(name="small", bufs=2))
        ysb_p = ctx.enter_context(tc.tile_pool(name="ysb", bufs=2))
        ps_s = ctx.enter_context(tc.tile_pool(name="ps_s", bufs=2, space="PSUM"))
        ps_o = ctx.enter_context(tc.tile_pool(name="ps_o", bufs=2, space="PSUM"))
        dram = ctx.enter_context(tc.tile_pool(name="dram", bufs=1, space="DRAM"))

        # ---- constants ----
        ident = const.tile([128, 128], f16)
        make_identity(nc, ident[:])
        ones_f = const.tile([128, 1], f32)
        nc.any.memset(ones_f[:], 1.0)
        ones_r = const.tile([1, 64], f32)    # bcast matmul stationary (kept fp32: exact)
        nc.any.memset(ones_r[:], 1.0)
        ones3 = const.tile([128, HPC], f32)  # Vones ones-columns source
        nc.any.memset(ones3[:], 1.0)

        # ---- persistent activations (f32r) ----
        QT0 = big.tile([128, S], f16)   # heads 0,1 of this core's group: Q^T
        QT1 = big.tile([64, S], f16)    # head 2: Q^T
        KT0 = big.tile([128, S], f16)   # heads 0,1: K^T
        KT1p = big.tile([128, S], f16)  # rows 0-63: head2 K^T ; rows 64-127: head2 outT (normalized)
        OT0 = big.tile([128, S], f16)   # normalized outT heads 0,1
        VON = big.tile([128, NST * HPC * 65], f16)  # per s-tile: [V_h|1] x3 interleaved

        # ---- weights -> SBUF f32r ----
        wq_r = big.tile([128, 6 * E], f16)
        wk_r = big.tile([128, 6 * E], f16)
        wv_r = big.tile([128, 6 * EP], f16)
        wo_r0 = big.tile([128, D], f16)           # woT rows 0-127
        wo_r1 = big.tile([128, D], f16)           # rows 64-127 hold woT rows 128-191

        for w_dram, w_sb, width in ((wqT, wq_r, E), (wkT, wk_r, E), (wvT, wv_r, EP)):
            for j in range(6):
                nc.gpsimd.dma_start(w_sb[:, j * width:(j + 1) * width],
                                    w_dram[j * 128:(j + 1) * 128, :])
        nc.gpsimd.dma_start(wo_r0[:], woT[0:128, :])
        nc.gpsimd.dma_start(wo_r1[64:128, :], woT[128:192, :])

        # ---- phase 1: transpose inputs + projections ----
        def proj_qk(x_dram, w_sb, dst0, dst1):
            # dst0 [128, S] (e rows 0-127), dst1 [64, S] (e rows 128-191)
            for c in range(NCH):
                strips = []
                xt = []
                for st in range(4):
                    t = xload_p.tile([128, D], f16, tag="xload")
                    nc.gpsimd.dma_start(t[:], x_dram[c * 512 + st * 128:c * 512 + (st + 1) * 128, :])
                    xt.append(t)
                for dt in range(6):
                    tp = ps_s.tile([128, 512], f16, tag="s")
                    for st in range(4):
                        nc.tensor.transpose(tp[:, st * 128:(st + 1) * 128],
                                            xt[st][:, dt * 128:(dt + 1) * 128], ident[:])
                    sb = strip_p.tile([128, 512], f16, tag="strip")
                    nc.vector.tensor_copy(sb[:], tp[:])
                    strips.append(sb)
                for ep, (lo, sz) in enumerate(((0, 128), (128, 64))):
                    pp = ps_o.tile([128, 512], f32, tag="o")
                    for dt in range(6):
                        nc.tensor.matmul(pp[0:sz, :],
                                         wq_like_slice(w_sb, dt, lo, sz),
                                         strips[dt][:],
                                         start=(dt == 0), stop=(dt == 5))
                    dst = dst0 if ep == 0 else dst1
                    nc.vector.tensor_copy(dst[0:sz, c * 512:(c + 1) * 512], pp[0:sz, :])

        def wq_like_slice(w_sb, dt, lo, sz):
            return w_sb[:, dt * E + lo:dt * E + lo + sz]

        proj_qk(xq, wq_r, QT0, QT1)
        proj_qk(xk, wk_r, KT0, KT1p)

        # V: natural layout with interleaved ones columns
        for c in range(NCH):
            strips = []
            xt = []
            for st in range(4):
                t = xload_p.tile([128, D], f16, tag="xload")
                nc.gpsimd.dma_start(t[:], xv[c * 512 + st * 128:c * 512 + (st + 1) * 128, :])
                xt.append(t)
            for dt in range(6):
                tp = ps_s.tile([128, 512], f16, tag="s")
                for st in range(4):
                    nc.tensor.transpose(tp[:, st * 128:(st + 1) * 128],
                                        xt[st][:, dt * 128:(dt + 1) * 128], ident[:])
                sb = strip_p.tile([128, 512], f16, tag="strip")
                nc.vector.tensor_copy(sb[:], tp[:])
                strips.append(sb)
            for st in range(4):
                pp = ps_o.tile([128, EP], f32, tag="o")
                for dt in range(6):
                    nc.tensor.matmul(pp[:], strips[dt][:, st * 128:(st + 1) * 128],
                                     wv_r[:, dt * EP:(dt + 1) * EP],
                                     start=(dt == 0), stop=(dt == 5))
                gst = c * 4 + st
                von = VON[:, gst * HPC * 65:(gst + 1) * HPC * 65]
                v3 = von.rearrange("p (h c) -> p h c", c=65)
                nc.vector.tensor_copy(v3[:, :, 0:64],
                                      pp[:, 0:E].rearrange("p (h c) -> p h c", c=64))
                nc.vector.tensor_copy(v3[:, :, 64:65],
                                      ones3[:].rearrange("p (h c) -> p h c", c=1))

        # ---- phase 2: attention (transposed domain) ----
        def head_aps(h):
            if h == 0:
                return KT0[0:64, :], QT0[0:64, :]
            if h == 1:
                return KT0[64:128, :], QT0[64:128, :]
            return KT1p[0:64, :], QT1[0:64, :]

        for qc in range(NQC):
            q0, q1 = qc * QC, (qc + 1) * QC
            for h in range(HPC):
                KT_h, QT_h = head_aps(h)
                po = ps_o.tile([65, QC], f32, tag="o")
                for t in range(NST):
                    pscore = ps_s.tile([128, QC], f32, tag="s")
                    for half in range(2):
                        nc.tensor.matmul(
                            pscore[:, half * 512:(half + 1) * 512],
                            KT_h[:, t * 128:(t + 1) * 128],
                            QT_h[:, q0 + half * 512:q0 + (half + 1) * 512],
                            start=True, stop=True)
                    et = expt_p.tile([128, QC], f16, tag="expt")
                    nc.scalar.activation(et[:], pscore[:], AF.Exp, scale=0.125)
                    von = VON[:, (t * HPC + h) * 65:(t * HPC + h + 1) * 65]
                    for half in range(2):
                        nc.tensor.matmul(
                            po[:, half * 512:(half + 1) * 512],
                            von, et[:, half * 512:(half + 1) * 512],
                            start=(t == 0), stop=(t == NST - 1))
                # normalize: outT[d, q] / denom[q]
                rc = small_p.tile([1, QC], f32, tag="recip")
                with nc.allow_low_precision(reason="softmax denominator"):
                    nc.vector.reciprocal(rc[:], po[64:65, :])
                pb = ps_s.tile([128, QC], f32, tag="s")
                for half in range(2):
                    nc.tensor.matmul(pb[0:64, half * 512:(half + 1) * 512],
                                     ones_r[:], rc[:, half * 512:(half + 1) * 512],
                                     start=True, stop=True)
                bs = small_p.tile([64, QC], f32, tag="bcast")
                nc.vector.tensor_copy(bs[:], pb[0:64, :])
                nrm = small_p.tile([64, QC], f16, tag="nrm")
                nc.vector.tensor_mul(nrm[:], po[0:64, :], bs[:])
                # move into packed destination (partition shift needs DMA)
                if h == 0:
                    nc.vector.tensor_copy(OT0[0:64, q0:q1], nrm[:])
                elif h == 1:
                    nc.sync.dma_start(OT0[64:128, q0:q1], nrm[:])
                else:
                    nc.sync.dma_start(KT1p[64:128, q0:q1], nrm[:])

        # ---- phase 3: W_o + ReduceScatter ----
        rs_in = dram.tile([S, D], f32)
        rs_out = dram.tile([S // 4, D], f32)
        for st in range(NST):
            py = ps_o.tile([128, D], f32, tag="o")
            for e0, esz in ((0, 512), (512, 256)):
                nc.tensor.matmul(py[:, e0:e0 + esz],
                                 OT0[:, st * 128:(st + 1) * 128],
                                 wo_r0[:, e0:e0 + esz], start=True, stop=False)
                nc.tensor.matmul(py[:, e0:e0 + esz],
                                 KT1p[64:128, st * 128:(st + 1) * 128],
                                 wo_r1[64:128, e0:e0 + esz], start=False, stop=True)
            ys = ysb_p.tile([128, D], f32, tag="ysb")
            nc.vector.tensor_copy(ys[:], py[:])
            nc.sync.dma_start(rs_in[st * 128:(st + 1) * 128, :], ys[:])

        nc.gpsimd.collective_compute(
            "ReduceScatter",
            mybir.AluOpType.add,
            replica_groups=[[0, 1, 2, 3], [4, 5, 6, 7]],
            ins=[rs_in.opt()],
            outs=[rs_out.opt()],
        )
        nc.sync.dma_start(y[:], rs_out[:])


_NC_CACHE = None


def _get_nc():
    global _NC_CACHE
    if _NC_CACHE is None:
        _NC_CACHE = _build_nc()
    return _NC_CACHE


def _make_in_maps(query, key, value, W_q, W_k, W_v, W_o):
    query = np.asarray(query, dtype=np.float32)
    key = np.asarray(key, dtype=np.float32)
    value = np.asarray(value, dtype=np.float32)
    wq_t = np.ascontiguousarray(np.asarray(W_q, np.float32).T)  # [d_in, e_out]
    wk_t = np.ascontiguousarray(np.asarray(W_k, np.float32).T)
    wv_t = np.ascontiguousarray(np.asarray(W_v, np.float32).T)
    wo_t = np.ascontiguousarray(np.asarray(W_o, np.float32).T)  # [d_in(heads), e_out]
    in_maps = []
    for c in range(NCORES):
        b, g = c // 4, c % 4
        sl = slice(g * E, (g + 1) * E)
        wv_pad = np.zeros((D, EP), np.float32)
        wv_pad[:, 0:E] = wv_t[:, sl]
        in_maps.append({
            "xq": np.ascontiguousarray(query[b]),
            "xk": np.ascontiguousarray(key[b]),
            "xv": np.ascontiguousarray(value[b]),
            "wqT": np.ascontiguousarray(wq_t[:, sl]),
            "wkT": np.ascontiguousarray(wk_t[:, sl]),
            "wvT": wv_pad,
            "woT": np.ascontiguousarray(wo_t[sl, :]),
        })
    return in_maps


def run(in_maps, trace=False):
    nc = _get_nc()
    return bass_utils.run_bass_kernel_spmd(
        nc, in_maps, core_ids=list(range(NCORES)), trace=trace)


def kernel(**inputs):
    in_maps = _make_in_maps(**inputs)
    res = run(in_maps)
    out = np.empty((B, S, D), np.float32)
    for c in range(NCORES):
        b, g = c // 4, c % 4
        out[b, g * (S // 4):(g + 1) * (S // 4)] = res.results[c]["y"]
    return out
